# revision 6
# baseline (speedup 1.0000x reference)
"""3-layer GCN (GCNConv x3, PyG-default norm) on 8 Trainium2 NeuronCores.

Single-launch dst-sharded design. Each core owns 12500 dst nodes laid out
degree-sorted into 98 blocks of 128 positions. Per layer, each core builds
its shard of the "table" (dinv[n]*(h[n] @ W) rows) with PE matmuls, the
shards are exchanged on-device with an 8-core AllGather collective, and
in-edge messages (+ self loop) are fetched per (block-group, quadrant)
with gpsimd dma_gather (int16 quadrant-local indices) and summed with
strided tensor_reduce. The epilogue applies dinv, bias, relu. All three
layers plus the exchanges run in ONE device launch; features ship as fp16
to cut host<->device transfer.
"""

import numpy as np

N = 100000
D = 64
NCORES = 8
NPC = N // NCORES
P = 128
NB = 98
SH = (NB + 1) * P            # 12672 rows/shard (incl zero block)
TBL = NCORES * SH            # 101376
QROWS = 2 * SH               # 25344 int16-addressable rows
PADLOC = NB * P              # zero-block row (shard-low of quadrant)
COLS_CAP = 128
GMAX = 8

_CACHE = {}


def _plan(edge_index):
    key = hash(edge_index.tobytes())
    if key in _CACHE:
        return _CACHE[key]
    src = np.asarray(edge_index[0], dtype=np.int64)
    dst = np.asarray(edge_index[1], dtype=np.int64)
    qcnt = np.bincount(dst * 4 + src // (2 * NPC), minlength=N * 4).reshape(N, 4)
    ar = np.arange(N)
    qcnt[ar, ar // (2 * NPC)] += 1                       # self-loop slot
    deg = qcnt.sum(1).astype(np.float64)
    dinv = (1.0 / np.sqrt(deg)).astype(np.float32)

    # positions sorted by max per-quadrant load: groups nodes with similar
    # dominant-quadrant counts into blocks, minimizing per-block gather pad
    sort_key = -(qcnt.max(1) * 100 + qcnt.sum(1))
    pos_of = np.empty(N, dtype=np.int64)
    pes = []
    for c in range(NCORES):
        own = np.arange(c * NPC, (c + 1) * NPC)
        order = np.argsort(sort_key[own], kind="stable")
        pos_of[own[order]] = np.arange(NPC)
        pes.append(own[order])

    # per-core edge lists (edges + self loops) and shared per-(block,q) maxima
    per_core = []
    Kbq = np.zeros((NB, 4), dtype=np.int64)
    for c in range(NCORES):
        m = (dst >= c * NPC) & (dst < (c + 1) * NPC)
        s_ = src[m]
        own = np.arange(c * NPC, (c + 1) * NPC)
        ps = np.concatenate([pos_of[dst[m]], pos_of[own]])
        ss = np.concatenate([s_, own])
        qq = ss // (2 * NPC)
        loc = ((ss // NPC) % 2) * SH + pos_of[ss]
        key_e = ps * 4 + qq
        o = np.argsort(key_e, kind="stable")
        sk = key_e[o]
        new = np.r_[True, sk[1:] != sk[:-1]]
        starts = np.flatnonzero(new)
        gid = np.cumsum(new) - 1
        kk = np.empty(len(sk), dtype=np.int64)
        kk[o] = np.arange(len(sk)) - starts[gid]
        cnt = np.bincount(key_e, minlength=NB * P * 4).reshape(NB, P, 4)
        Kbq = np.maximum(Kbq, cnt.max(1))
        per_core.append((ps, qq, kk, loc))

    groups = []
    b = 0
    while b < NB:
        G = 1
        K = Kbq[b].copy()
        while b + G < NB and G < GMAX:
            K2 = np.maximum(K, Kbq[b + G])
            if (G + 1) * int(K2.sum()) > COLS_CAP:
                break
            K = K2
            G += 1
        groups.append((b, G, K.astype(np.int64)))
        b += G

    calls = []
    c16 = 0
    for gi, (bs_, G, K) in enumerate(groups):
        qoff = 0
        for q in range(4):
            Kq = int(K[q])
            if Kq == 0:
                continue
            n = P * G * Kq
            calls.append((gi, q, c16, n, qoff))
            c16 += n // 16
            qoff += G * Kq
    TOTC16 = c16

    # slot-base lookup tables indexed by (group, quadrant)
    gstart = np.empty(NB, dtype=np.int64)
    grp_of = np.empty(NB, dtype=np.int64)
    for gi, (bs_, G, K) in enumerate(groups):
        grp_of[bs_:bs_ + G] = gi
        gstart[bs_:bs_ + G] = bs_
    base_gq = np.full((len(groups), 4), -1, dtype=np.int64)
    K_gq = np.zeros((len(groups), 4), dtype=np.int64)
    for (gi, q, c16s, n, qoff) in calls:
        base_gq[gi, q] = c16s * 16
        K_gq[gi, q] = (n // P) // len(range(groups[gi][0], groups[gi][0] + groups[gi][1]))

    idx16 = []
    for c in range(NCORES):
        ps, qq, kk, loc = per_core[c]
        bb = ps >> 7
        pp = ps & 127
        gi_e = grp_of[bb]
        g_e = bb - gstart[bb]
        Kq_e = K_gq[gi_e, qq]
        slot = base_gq[gi_e, qq] + (g_e * Kq_e + kk) * P + pp
        flat = np.full(TOTC16 * 16, PADLOC, dtype=np.int16)
        flat[slot] = loc.astype(np.int16)
        idx16.append(np.ascontiguousarray(flat.reshape(-1, 16).T))

    dinvbs = []
    for c in range(NCORES):
        dv = np.zeros(NB * P, dtype=np.float32)
        dv[:NPC] = dinv[pes[c]]
        dinvbs.append(np.ascontiguousarray(dv.reshape(NB, P).T))

    plan = dict(dinv=dinv, pes=pes, groups=groups, calls=calls,
                idx16=idx16, dinvbs=dinvbs, TOTC16=TOTC16)
    _CACHE[key] = plan
    return plan


def _build(plan):
    """One NEFF: L0 table build, then 3x (AllGather, gather, reduce,
    epilogue) with layer-2/3 table builds via PE transpose + matmul."""
    import concourse.bacc as bacc
    import concourse.mybir as mybir
    import concourse.tile as tile
    from concourse.masks import make_identity

    groups, calls, TOTC16 = plan["groups"], plan["calls"], plan["TOTC16"]
    f32 = mybir.dt.float32
    f16 = mybir.dt.float16
    i16 = mybir.dt.int16
    nc = bacc.Bacc("TRN2", target_bir_lowering=False, num_swdge_queues=2)
    xt_in = nc.dram_tensor("xt16", [D, SH], f16, kind="ExternalInput")
    idx_in = nc.dram_tensor("idx16", [16, TOTC16], i16, kind="ExternalInput")
    dinv_in = nc.dram_tensor("dinvb", [P, NB], f32, kind="ExternalInput")
    bias_in = nc.dram_tensor("bias3", [P, 3 * D], f32, kind="ExternalInput")
    w_in = nc.dram_tensor("w16", [D, 3 * D], f16, kind="ExternalInput")
    h_out = nc.dram_tensor("h16", [NB * P, D], f16, kind="ExternalOutput")

    with tile.TileContext(nc) as tc:
        with (
            tc.tile_pool(name="cst", bufs=1) as cst,
            tc.tile_pool(name="wk", bufs=2) as wk,
            tc.tile_pool(name="ep", bufs=2) as ep,
            tc.tile_pool(name="st", bufs=2) as st,
            tc.tile_pool(name="psT", bufs=2, space="PSUM") as psT,
            tc.tile_pool(name="psM", bufs=2, space="PSUM") as psM,
            tc.tile_pool(name="dram", bufs=1, space="DRAM") as dram,
        ):
            tloc = [dram.tile([SH, D], f32, name=f"tloc{i}") for i in range(3)]
            table = [dram.tile([TBL, D], f32, addr_space="Shared",
                               name=f"table{i}") for i in range(3)]

            idx_sb = cst.tile([P, TOTC16], i16)
            for k in range(8):
                nc.sync.dma_start(out=idx_sb[16 * k:16 * (k + 1), :], in_=idx_in[:])
            dinvb = cst.tile([P, NB], f32)
            nc.sync.dma_start(out=dinvb[:], in_=dinv_in[:])
            bias3 = cst.tile([P, 3 * D], f32)
            nc.sync.dma_start(out=bias3[:], in_=bias_in[:])
            w16 = cst.tile([D, 3 * D], f16)
            nc.sync.dma_start(out=w16[:], in_=w_in[:])
            xt16 = cst.tile([D, SH], f16)
            nc.sync.dma_start(out=xt16[:], in_=xt_in[:])
            ident = cst.tile([P, P], f32)
            make_identity(nc, ident[:])
            zb = cst.tile([P, D], f32)
            nc.vector.memset(zb[:], 0.0)
            nc.sync.dma_start(out=tloc[1][NB * P:SH, :], in_=zb[:])
            nc.sync.dma_start(out=tloc[2][NB * P:SH, :], in_=zb[:])

            # L0: tloc[0] rows = (dinv*x) @ W1  (xt16 pre-scaled/transposed)
            for g0 in range(0, NB + 1, 8):
                gn = min(8, NB + 1 - g0)
                pst = psM.tile([P, 8 * D], f32, tag="ps0")
                for j in range(gn):
                    blk = g0 + j
                    nc.tensor.matmul(
                        out=pst[:, j * D:(j + 1) * D],
                        lhsT=xt16[:, blk * P:(blk + 1) * P],
                        rhs=w16[:, 0:D], start=True, stop=True)
                stg = st.tile([P, 8 * D], f32, tag="stg0")
                nc.vector.tensor_copy(out=stg[:, :gn * D], in_=pst[:, :gn * D])
                nc.sync.dma_start(
                    out=tloc[0][g0 * P:(g0 + gn) * P, :]
                        .rearrange("(g p) d -> p g d", p=P),
                    in_=stg[:, :gn * D])

            for lyr in range(3):
                last = lyr == 2
                nc.gpsimd.collective_compute(
                    "AllGather",
                    mybir.AluOpType.bypass,
                    replica_groups=[list(range(NCORES))],
                    ins=[tloc[lyr].opt()],
                    outs=[table[lyr].opt()],
                )
                tbl = table[lyr]
                for gi, (bstart, G, K) in enumerate(groups):
                    COLS = G * int(K.sum())
                    gbuf = wk.tile([P, COLS, D], f32, tag="gbuf")
                    for (gi2, q, c16s, n, qoff) in calls:
                        if gi2 != gi:
                            continue
                        nc.gpsimd.dma_gather(
                            out_ap=gbuf[:, qoff:qoff + n // P, :],
                            in_ap=tbl[q * QROWS:(q + 1) * QROWS, :],
                            idxs_ap=idx_sb[:, c16s:c16s + n // 16],
                            num_idxs=n, num_idxs_reg=n, elem_size=D,
                            single_packet=False, queue_num=q % 2)
                    acc = ep.tile([P, GMAX, D], f32, tag="acc")
                    tmp = ep.tile([P, GMAX, D], f32, tag="tmp")
                    first = True
                    for (gi2, q, c16s, n, qoff) in calls:
                        if gi2 != gi:
                            continue
                        Kq = (n // P) // G
                        red_in = gbuf[:, qoff:qoff + G * Kq, :] \
                            .rearrange("p (g k) d -> p g d k", g=G)
                        nc.vector.tensor_reduce(
                            out=(acc if first else tmp)[:, :G, :], in_=red_in,
                            axis=mybir.AxisListType.X, op=mybir.AluOpType.add)
                        if not first:
                            nc.vector.tensor_tensor(
                                out=acc[:, :G, :], in0=acc[:, :G, :],
                                in1=tmp[:, :G, :], op=mybir.AluOpType.add)
                        first = False
                    dvb = dinvb[:, bstart:bstart + G].to_broadcast([P, G, D])
                    bias = bias3[:, lyr * D:(lyr + 1) * D] \
                        .rearrange("p (g d) -> p g d", g=1).to_broadcast([P, G, D])
                    t1 = ep.tile([P, GMAX, D], f32, tag="t1")
                    nc.vector.tensor_tensor(out=t1[:, :G, :], in0=acc[:, :G, :],
                                            in1=dvb, op=mybir.AluOpType.mult)
                    t2 = ep.tile([P, GMAX, D], f32, tag="t2")
                    nc.vector.tensor_tensor(out=t2[:, :G, :], in0=t1[:, :G, :],
                                            in1=bias, op=mybir.AluOpType.add)
                    h = ep.tile([P, GMAX, D], f32, tag="h")
                    nc.scalar.activation(out=h[:, :G, :], in_=t2[:, :G, :],
                                         func=mybir.ActivationFunctionType.Relu)
                    if last:
                        h16s = ep.tile([P, GMAX, D], f16, tag="h16s")
                        nc.vector.tensor_copy(out=h16s[:, :G, :], in_=h[:, :G, :])
                        nc.sync.dma_start(
                            out=h_out[bstart * P:(bstart + G) * P, :]
                                .rearrange("(g p) d -> p g d", p=P),
                            in_=h16s[:, :G, :])
                    else:
                        hh = ep.tile([P, GMAX, D], f32, tag="hh")
                        nc.vector.tensor_tensor(out=hh[:, :G, :], in0=h[:, :G, :],
                                                in1=dvb, op=mybir.AluOpType.mult)
                        agst = ep.tile([P, GMAX, D], f32, tag="agst")
                        for bqi in range(G):
                            pt = psT.tile([D, P], f32, tag="pt")
                            nc.tensor.transpose(out=pt[:], in_=hh[:, bqi, :],
                                                identity=ident[:])
                            ht = ep.tile([D, P], f16, tag="ht")
                            nc.scalar.copy(out=ht[:], in_=pt[:])
                            pm = psM.tile([P, D], f32, tag="pm")
                            nc.tensor.matmul(
                                out=pm[:], lhsT=ht[:],
                                rhs=w16[:, (lyr + 1) * D:(lyr + 2) * D],
                                start=True, stop=True)
                            nc.vector.tensor_copy(out=agst[:, bqi, :], in_=pm[:])
                        nc.sync.dma_start(
                            out=tloc[lyr + 1][bstart * P:(bstart + G) * P, :]
                                .rearrange("(g p) d -> p g d", p=P),
                            in_=agst[:, :G, :])
    nc.compile()
    return nc


def _warm_devices():
    """Establish the device connection before timing: the first contact
    with an idle axon terminal pays a multi-second reclaim penalty that
    is unrelated to the kernel itself."""
    import jax
    jax.block_until_ready(jax.device_put(np.zeros(8, np.float32), jax.devices()[0]))


def kernel(x, W1, b1, W2, b2, W3, b3, edge_index):
    import time as _t
    from concourse.bass_utils import run_bass_kernel_spmd as _rb

    x = np.asarray(x, dtype=np.float32)
    Ws = [np.asarray(w, dtype=np.float32) for w in (W1, W2, W3)]
    bs = [np.asarray(b, dtype=np.float32) for b in (b1, b2, b3)]
    plan = _plan(np.asarray(edge_index))
    dinv, pes, dinvbs = plan["dinv"], plan["pes"], plan["dinvbs"]
    cores = list(range(NCORES))

    if "nc" not in plan:
        plan["nc"] = _build(plan)
    nc = plan["nc"]
    _warm_devices()

    bias3 = np.ascontiguousarray(
        np.tile(np.concatenate(bs)[None, :], (P, 1)).astype(np.float32))
    w16 = np.ascontiguousarray(
        np.concatenate(Ws, axis=1).astype(np.float16))
    in_maps = []
    for c in cores:
        pe = pes[c]
        xt = np.zeros((D, SH), dtype=np.float16)
        xt[:, :NPC] = (x[pe] * dinv[pe][:, None]).T.astype(np.float16)
        in_maps.append(dict(xt16=xt, idx16=plan["idx16"][c],
                            dinvb=dinvbs[c], bias3=bias3, w16=w16))

    t0 = _t.time()
    r = _rb(nc, in_maps, core_ids=cores)
    wall_ns = (_t.time() - t0) * 1e9
    t_ns = r.exec_time_ns if r.exec_time_ns is not None else int(wall_ns)

    out = np.empty((N, D), dtype=np.float32)
    for c in cores:
        out[pes[c]] = r.results[c]["h16"][:NPC].astype(np.float32)
    print(f"HW exec time: {t_ns} ns")
    return out


# revision 7
# speedup vs baseline: 1.5143x; 1.5143x over previous
"""3-layer GCN (GCNConv x3, PyG-default norm) on 8 Trainium2 NeuronCores.

Single-launch dst-sharded design. Each core owns 12500 dst nodes laid out
degree-sorted into 98 blocks of 128 positions. Per layer, each core builds
its shard of the "table" (dinv[n]*(h[n] @ W) rows) with PE matmuls, the
shards are exchanged on-device with an 8-core AllGather collective, and
in-edge messages (+ self loop) are fetched per (block-group, quadrant)
with gpsimd dma_gather (int16 quadrant-local indices) and summed with
strided tensor_reduce. The epilogue applies dinv, bias, relu. All three
layers plus the exchanges run in ONE device launch; features ship as fp16
to cut host<->device transfer.
"""

import numpy as np

N = 100000
D = 64
NCORES = 8
NPC = N // NCORES
P = 128
NB = 98
SH = (NB + 1) * P            # 12672 rows/shard (incl zero block)
TBL = NCORES * SH            # 101376
QROWS = 2 * SH               # 25344 int16-addressable rows
PADLOC = NB * P              # zero-block row (shard-low of quadrant)
COLS_CAP = 128
GMAX = 8

_CACHE = {}


def _plan(edge_index):
    key = hash(edge_index.tobytes())
    if key in _CACHE:
        return _CACHE[key]
    src = np.asarray(edge_index[0], dtype=np.int64)
    dst = np.asarray(edge_index[1], dtype=np.int64)
    qcnt = np.bincount(dst * 4 + src // (2 * NPC), minlength=N * 4).reshape(N, 4)
    ar = np.arange(N)
    qcnt[ar, ar // (2 * NPC)] += 1                       # self-loop slot
    deg = qcnt.sum(1).astype(np.float64)
    dinv = (1.0 / np.sqrt(deg)).astype(np.float32)

    # positions sorted by max per-quadrant load: groups nodes with similar
    # dominant-quadrant counts into blocks, minimizing per-block gather pad
    sort_key = -(qcnt.max(1) * 100 + qcnt.sum(1))
    pos_of = np.empty(N, dtype=np.int64)
    pes = []
    for c in range(NCORES):
        own = np.arange(c * NPC, (c + 1) * NPC)
        order = np.argsort(sort_key[own], kind="stable")
        pos_of[own[order]] = np.arange(NPC)
        pes.append(own[order])

    # per-core edge lists (edges + self loops) and shared per-(block,q) maxima
    per_core = []
    Kbq = np.zeros((NB, 4), dtype=np.int64)
    for c in range(NCORES):
        m = (dst >= c * NPC) & (dst < (c + 1) * NPC)
        s_ = src[m]
        own = np.arange(c * NPC, (c + 1) * NPC)
        ps = np.concatenate([pos_of[dst[m]], pos_of[own]])
        ss = np.concatenate([s_, own])
        qq = ss // (2 * NPC)
        loc = ((ss // NPC) % 2) * SH + pos_of[ss]
        key_e = ps * 4 + qq
        o = np.argsort(key_e, kind="stable")
        sk = key_e[o]
        new = np.r_[True, sk[1:] != sk[:-1]]
        starts = np.flatnonzero(new)
        gid = np.cumsum(new) - 1
        kk = np.empty(len(sk), dtype=np.int64)
        kk[o] = np.arange(len(sk)) - starts[gid]
        cnt = np.bincount(key_e, minlength=NB * P * 4).reshape(NB, P, 4)
        Kbq = np.maximum(Kbq, cnt.max(1))
        per_core.append((ps, qq, kk, loc))

    groups = []
    b = 0
    while b < NB:
        G = 1
        K = Kbq[b].copy()
        while b + G < NB and G < GMAX:
            K2 = np.maximum(K, Kbq[b + G])
            if (G + 1) * int(K2.sum()) > COLS_CAP:
                break
            K = K2
            G += 1
        groups.append((b, G, K.astype(np.int64)))
        b += G

    calls = []
    c16 = 0
    for gi, (bs_, G, K) in enumerate(groups):
        qoff = 0
        for q in range(4):
            Kq = int(K[q])
            if Kq == 0:
                continue
            n = P * G * Kq
            calls.append((gi, q, c16, n, qoff))
            c16 += n // 16
            qoff += G * Kq
    TOTC16 = c16

    # slot-base lookup tables indexed by (group, quadrant)
    gstart = np.empty(NB, dtype=np.int64)
    grp_of = np.empty(NB, dtype=np.int64)
    for gi, (bs_, G, K) in enumerate(groups):
        grp_of[bs_:bs_ + G] = gi
        gstart[bs_:bs_ + G] = bs_
    base_gq = np.full((len(groups), 4), -1, dtype=np.int64)
    K_gq = np.zeros((len(groups), 4), dtype=np.int64)
    for (gi, q, c16s, n, qoff) in calls:
        base_gq[gi, q] = c16s * 16
        K_gq[gi, q] = (n // P) // len(range(groups[gi][0], groups[gi][0] + groups[gi][1]))

    idx16 = []
    for c in range(NCORES):
        ps, qq, kk, loc = per_core[c]
        bb = ps >> 7
        pp = ps & 127
        gi_e = grp_of[bb]
        g_e = bb - gstart[bb]
        Kq_e = K_gq[gi_e, qq]
        slot = base_gq[gi_e, qq] + (g_e * Kq_e + kk) * P + pp
        flat = np.full(TOTC16 * 16, PADLOC, dtype=np.int16)
        flat[slot] = loc.astype(np.int16)
        idx16.append(np.ascontiguousarray(flat.reshape(-1, 16).T))

    dinvbs = []
    for c in range(NCORES):
        dv = np.zeros(NB * P, dtype=np.float32)
        dv[:NPC] = dinv[pes[c]]
        dinvbs.append(np.ascontiguousarray(dv.reshape(NB, P).T))

    plan = dict(dinv=dinv, pes=pes, groups=groups, calls=calls,
                idx16=idx16, dinvbs=dinvbs, TOTC16=TOTC16)
    _CACHE[key] = plan
    return plan


def _build(plan):
    """One NEFF: L0 table build, then 3x (AllGather, gather, reduce,
    epilogue) with layer-2/3 table builds via PE transpose + matmul."""
    import concourse.bacc as bacc
    import concourse.mybir as mybir
    import concourse.tile as tile
    from concourse.masks import make_identity

    groups, calls, TOTC16 = plan["groups"], plan["calls"], plan["TOTC16"]
    f32 = mybir.dt.float32
    f16 = mybir.dt.float16
    i16 = mybir.dt.int16
    nc = bacc.Bacc("TRN2", target_bir_lowering=False, num_swdge_queues=2)
    xt_in = nc.dram_tensor("xt16", [D, SH], f16, kind="ExternalInput")
    idx_in = nc.dram_tensor("idx16", [16, TOTC16], i16, kind="ExternalInput")
    dinv_in = nc.dram_tensor("dinvb", [P, NB], f32, kind="ExternalInput")
    bias_in = nc.dram_tensor("bias3", [P, 3 * D], f32, kind="ExternalInput")
    w_in = nc.dram_tensor("w16", [D, 3 * D], f16, kind="ExternalInput")
    h_out = nc.dram_tensor("h16", [NB * P, D], f16, kind="ExternalOutput")

    with tile.TileContext(nc) as tc:
        with (
            tc.tile_pool(name="cst", bufs=1) as cst,
            tc.tile_pool(name="wk", bufs=2) as wk,
            tc.tile_pool(name="ep", bufs=2) as ep,
            tc.tile_pool(name="st", bufs=2) as st,
            tc.tile_pool(name="psT", bufs=2, space="PSUM") as psT,
            tc.tile_pool(name="psM", bufs=2, space="PSUM") as psM,
            tc.tile_pool(name="dram", bufs=1, space="DRAM") as dram,
        ):
            tloc = [dram.tile([SH, D], f32, name=f"tloc{i}") for i in range(3)]
            table = [dram.tile([TBL, D], f32, addr_space="Shared",
                               name=f"table{i}") for i in range(3)]

            idx_sb = cst.tile([P, TOTC16], i16)
            for k in range(8):
                nc.sync.dma_start(out=idx_sb[16 * k:16 * (k + 1), :], in_=idx_in[:])
            dinvb = cst.tile([P, NB], f32)
            nc.sync.dma_start(out=dinvb[:], in_=dinv_in[:])
            bias3 = cst.tile([P, 3 * D], f32)
            nc.sync.dma_start(out=bias3[:], in_=bias_in[:])
            w16 = cst.tile([D, 3 * D], f16)
            nc.sync.dma_start(out=w16[:], in_=w_in[:])
            xt16 = cst.tile([D, SH], f16)
            nc.sync.dma_start(out=xt16[:], in_=xt_in[:])
            ident = cst.tile([P, P], f32)
            make_identity(nc, ident[:])
            zb = cst.tile([P, D], f32)
            nc.vector.memset(zb[:], 0.0)
            nc.sync.dma_start(out=tloc[1][NB * P:SH, :], in_=zb[:])
            nc.sync.dma_start(out=tloc[2][NB * P:SH, :], in_=zb[:])

            # L0: tloc[0] rows = (dinv*x) @ W1  (xt16 pre-scaled/transposed)
            for g0 in range(0, NB + 1, 8):
                gn = min(8, NB + 1 - g0)
                pst = psM.tile([P, 8 * D], f32, tag="ps0")
                for j in range(gn):
                    blk = g0 + j
                    nc.tensor.matmul(
                        out=pst[:, j * D:(j + 1) * D],
                        lhsT=xt16[:, blk * P:(blk + 1) * P],
                        rhs=w16[:, 0:D], start=True, stop=True)
                stg = st.tile([P, 8 * D], f32, tag="stg0")
                nc.vector.tensor_copy(out=stg[:, :gn * D], in_=pst[:, :gn * D])
                nc.sync.dma_start(
                    out=tloc[0][g0 * P:(g0 + gn) * P, :]
                        .rearrange("(g p) d -> p g d", p=P),
                    in_=stg[:, :gn * D])

            for lyr in range(3):
                last = lyr == 2
                nc.gpsimd.collective_compute(
                    "AllGather",
                    mybir.AluOpType.bypass,
                    replica_groups=[list(range(NCORES))],
                    ins=[tloc[lyr].opt()],
                    outs=[table[lyr].opt()],
                )
                tbl = table[lyr]
                for gi, (bstart, G, K) in enumerate(groups):
                    COLS = G * int(K.sum())
                    gbuf = wk.tile([P, COLS, D], f32, tag="gbuf")
                    for (gi2, q, c16s, n, qoff) in calls:
                        if gi2 != gi:
                            continue
                        nc.gpsimd.dma_gather(
                            out_ap=gbuf[:, qoff:qoff + n // P, :],
                            in_ap=tbl[q * QROWS:(q + 1) * QROWS, :],
                            idxs_ap=idx_sb[:, c16s:c16s + n // 16],
                            num_idxs=n, num_idxs_reg=n, elem_size=D,
                            single_packet=False, queue_num=q % 2)
                    acc = ep.tile([P, GMAX, D], f32, tag="acc")
                    tmp = ep.tile([P, GMAX, D], f32, tag="tmp")
                    first = True
                    for (gi2, q, c16s, n, qoff) in calls:
                        if gi2 != gi:
                            continue
                        Kq = (n // P) // G
                        red_in = gbuf[:, qoff:qoff + G * Kq, :] \
                            .rearrange("p (g k) d -> p g d k", g=G)
                        nc.vector.tensor_reduce(
                            out=(acc if first else tmp)[:, :G, :], in_=red_in,
                            axis=mybir.AxisListType.X, op=mybir.AluOpType.add)
                        if not first:
                            nc.vector.tensor_tensor(
                                out=acc[:, :G, :], in0=acc[:, :G, :],
                                in1=tmp[:, :G, :], op=mybir.AluOpType.add)
                        first = False
                    dvb = dinvb[:, bstart:bstart + G].to_broadcast([P, G, D])
                    bias = bias3[:, lyr * D:(lyr + 1) * D] \
                        .rearrange("p (g d) -> p g d", g=1).to_broadcast([P, G, D])
                    t1 = ep.tile([P, GMAX, D], f32, tag="t1")
                    nc.vector.tensor_tensor(out=t1[:, :G, :], in0=acc[:, :G, :],
                                            in1=dvb, op=mybir.AluOpType.mult)
                    t2 = ep.tile([P, GMAX, D], f32, tag="t2")
                    nc.vector.tensor_tensor(out=t2[:, :G, :], in0=t1[:, :G, :],
                                            in1=bias, op=mybir.AluOpType.add)
                    h = ep.tile([P, GMAX, D], f32, tag="h")
                    nc.scalar.activation(out=h[:, :G, :], in_=t2[:, :G, :],
                                         func=mybir.ActivationFunctionType.Relu)
                    if last:
                        h16s = ep.tile([P, GMAX, D], f16, tag="h16s")
                        nc.vector.tensor_copy(out=h16s[:, :G, :], in_=h[:, :G, :])
                        nc.sync.dma_start(
                            out=h_out[bstart * P:(bstart + G) * P, :]
                                .rearrange("(g p) d -> p g d", p=P),
                            in_=h16s[:, :G, :])
                    else:
                        hh = ep.tile([P, GMAX, D], f32, tag="hh")
                        nc.vector.tensor_tensor(out=hh[:, :G, :], in0=h[:, :G, :],
                                                in1=dvb, op=mybir.AluOpType.mult)
                        agst = ep.tile([P, GMAX, D], f32, tag="agst")
                        for bqi in range(G):
                            pt = psT.tile([D, P], f32, tag="pt")
                            nc.tensor.transpose(out=pt[:], in_=hh[:, bqi, :],
                                                identity=ident[:])
                            ht = ep.tile([D, P], f16, tag="ht")
                            nc.scalar.copy(out=ht[:], in_=pt[:])
                            pm = psM.tile([P, D], f32, tag="pm")
                            nc.tensor.matmul(
                                out=pm[:], lhsT=ht[:],
                                rhs=w16[:, (lyr + 1) * D:(lyr + 2) * D],
                                start=True, stop=True)
                            nc.vector.tensor_copy(out=agst[:, bqi, :], in_=pm[:])
                        nc.sync.dma_start(
                            out=tloc[lyr + 1][bstart * P:(bstart + G) * P, :]
                                .rearrange("(g p) d -> p g d", p=P),
                            in_=agst[:, :G, :])
    nc.compile()
    return nc


def _warm_devices():
    """Establish the device connection before timing: the first contact
    with an idle axon terminal pays a multi-second reclaim penalty that
    is unrelated to the kernel itself."""
    import jax
    jax.block_until_ready(jax.device_put(np.zeros(8, np.float32), jax.devices()[0]))


def kernel(x, W1, b1, W2, b2, W3, b3, edge_index):
    import time as _t
    from concourse.bass_utils import run_bass_kernel_spmd as _rb

    x = np.asarray(x, dtype=np.float32)
    Ws = [np.asarray(w, dtype=np.float32) for w in (W1, W2, W3)]
    bs = [np.asarray(b, dtype=np.float32) for b in (b1, b2, b3)]
    plan = _plan(np.asarray(edge_index))
    dinv, pes, dinvbs = plan["dinv"], plan["pes"], plan["dinvbs"]
    cores = list(range(NCORES))

    if "nc" not in plan:
        plan["nc"] = _build(plan)
    nc = plan["nc"]
    _warm_devices()

    bias3 = np.ascontiguousarray(
        np.tile(np.concatenate(bs)[None, :], (P, 1)).astype(np.float32))
    w16 = np.ascontiguousarray(
        np.concatenate(Ws, axis=1).astype(np.float16))
    in_maps = []
    for c in cores:
        pe = pes[c]
        xt = np.zeros((D, SH), dtype=np.float16)
        xt[:, :NPC] = (x[pe] * dinv[pe][:, None]).T.astype(np.float16)
        in_maps.append(dict(xt16=xt, idx16=plan["idx16"][c],
                            dinvb=dinvbs[c], bias3=bias3, w16=w16))

    r = None
    last_err = None
    for attempt in range(3):
        try:
            if attempt:
                _t.sleep(5)
                _warm_devices()
            t0 = _t.time()
            r = _rb(nc, in_maps, core_ids=cores)
            break
        except Exception as e:          # transient device faults recover on retry
            last_err = e
    if r is None:
        raise last_err
    wall_ns = (_t.time() - t0) * 1e9
    t_ns = r.exec_time_ns if r.exec_time_ns is not None else int(wall_ns)

    out = np.empty((N, D), dtype=np.float32)
    for c in cores:
        out[pes[c]] = r.results[c]["h16"][:NPC].astype(np.float32)
    print(f"HW exec time: {t_ns} ns")
    return out
